# revision 37
# baseline (speedup 1.0000x reference)
"""Causal self-attention (B=2, S=2048, E=1024, H=16, D=64) on 8 TRN2 cores.

Sharding: core c = (batch b = c//4, head-group g = c%4) owns batch b and
heads 4g..4g+3 (a 256-wide slice of the QKV projections / Wo rows).
Each core computes its partial out-projection y_c = attout_c @ Wo_c; the
host sums the 4 partials per batch and adds the folded bias (bk drops out
of softmax; bv contributes bv @ Wo since softmax weights sum to 1).

Row-dependent precision (validated: end-to-end rel err ~8e-3 vs 2e-2 gate):
  - queries 0-511 (qc=0): bf16 pipeline. Early rows have concentrated
    softmax and O(1)-magnitude attout, so they need bf16.
  - queries 512+ (qc=1..3): fp8e4m3 pipeline with DoubleRow (DR) matmuls.
    attout magnitude ~1.65/sqrt(L) for context length L, so fp8's ~4%
    relative noise stays far below the absolute error budget.
  DR matmul: lhsT [K,2,M] fp8, rhs [K,2,N] fp8 -> out[M,N] = sum_g
  lhsT[:,g].T @ rhs[:,g], at 0.5 PE cycles/output-column (4x bf16
  throughput per contraction element). DR free-dim group strides must be
  16-byte aligned -> V head slots padded to 80 cols.

Layouts:
  Q^T/K^T [128, 2, S]: partition p = head(p//32)*32 + d%32, free dim g =
  d//32 (the 32+32 d-split lets scores contract d=64 as DR [32,2]).
  V [128, rt, 320]: natural keys-on-partitions; per head h cols h*80..+63
  are V, col h*80+64 is ones (PV row 64 = softmax denominator).
  attout^T [128, 2, S]: partitions (h%2)*64+d, group h//2 -- matches the
  Wo row packing so out-proj is a single DR matmul per (qt, nh).

exp runs on ACT (the end-to-end bottleneck: ~58us of causal-area exp), in
[128, 2hp, 512-off] tiles; fp8-path exp folds scale 1/8 and bias -1.5
(softmax-invariant shift that keeps exp below e4m3's 240 max). The causal
mask multiply runs on GPSIMD for the fp8 path (SBUF-only engine), DVE for
the bf16 path. PE idle during exp is filled with the next chunk's
projections / previous chunks' out-projections (baseline's filler
interleave, engine queues are in-order).
"""

import numpy as np

B, S, E, H = 2, 2048, 1024, 16
D = E // H          # 64
NCORES = 8
HPC = 4             # heads per core
HD = HPC * D        # 256 cols per core
KT = E // 128       # 8 contraction tiles
QC = S // 512       # 4 query chunks
NQT = S // 128      # 16 row tiles
V8W = HPC * 80      # 320: fp8 V with 80-wide head slots (16B-aligned)
VBW = HPC * (D + 1)  # 260: bf16 V with ones column per head
SHIFT = 1.5         # exp bias: exp(s/8 - SHIFT), cancels in softmax

_prog = None
LAST_RESULTS = None


def _build_program():
    import concourse.mybir as mybir
    import concourse.tile as tile
    from concourse import bacc, library_config

    f32 = mybir.dt.float32
    bf16 = mybir.dt.bfloat16
    fp8 = mybir.dt.float8e4
    Exp = mybir.ActivationFunctionType.Exp
    DR = mybir.MatmulPerfMode.DoubleRow

    nc = bacc.Bacc(trn_type="TRN2", target_bir_lowering=False, debug=False)

    xtb = nc.dram_tensor("xtb", [128, KT * 512], bf16, kind="ExternalInput").ap()
    xt8 = nc.dram_tensor("xt8", [128, 3 * KT * 512], fp8, kind="ExternalInput").ap()
    wqb = nc.dram_tensor("wqb", [128, KT * 2 * 128], bf16, kind="ExternalInput").ap()
    wkb = nc.dram_tensor("wkb", [128, KT * 2 * 128], bf16, kind="ExternalInput").ap()
    wq8 = nc.dram_tensor("wq8", [128, 8 * 2 * 128], fp8, kind="ExternalInput").ap()
    wk8 = nc.dram_tensor("wk8", [128, 8 * 2 * 128], fp8, kind="ExternalInput").ap()
    wvb = nc.dram_tensor("wvb", [128, KT * 256], bf16, kind="ExternalInput").ap()
    wv8 = nc.dram_tensor("wv8", [128, 4 * 2 * 256], fp8, kind="ExternalInput").ap()
    wob = nc.dram_tensor("wob", [128, 2 * E], bf16, kind="ExternalInput").ap()
    wo8 = nc.dram_tensor("wo8", [128, 2 * E], fp8, kind="ExternalInput").ap()
    bq = nc.dram_tensor("bqc", [128, 2], f32, kind="ExternalInput").ap()
    # lower-triangular band mask (valid iff q_local >= k), bf16 + fp8 copies
    maskb = nc.dram_tensor("maskb", [128, 128], bf16, kind="ExternalInput").ap()
    mask8 = nc.dram_tensor("mask8", [128, 128], fp8, kind="ExternalInput").ap()
    y = nc.dram_tensor("y", [128, NQT * E], bf16, kind="ExternalOutput").ap()

    with tile.TileContext(nc) as tc:
        with (
            tc.tile_pool(name="consts", bufs=1) as consts,
            tc.tile_pool(name="exps", bufs=8) as exps,
            tc.tile_pool(name="small", bufs=4) as small,
            tc.tile_pool(name="ps_sc", bufs=2, space="PSUM") as ps_sc,
            tc.tile_pool(name="ps_pj", bufs=2, space="PSUM") as ps_pj,
            tc.tile_pool(name="ps_acc", bufs=2, space="PSUM") as ps_acc,
        ):
            # ---- SBUF constants; DMA issue order = need order ----
            xtb_sb = consts.tile([128, KT, 512], bf16)
            xt8_sb = consts.tile([128, 3, KT, 512], fp8)
            wqb_sb = consts.tile([128, 2, KT, 128], bf16)
            wkb_sb = consts.tile([128, 2, KT, 128], bf16)
            wq8_sb = consts.tile([128, 8, 2, 128], fp8)
            wk8_sb = consts.tile([128, 8, 2, 128], fp8)
            wvb_sb = consts.tile([128, KT, 256], bf16)
            wv8_sb = consts.tile([128, 4, 2, 256], fp8)
            wob_sb = consts.tile([128, 2, E], bf16)
            wo8_sb = consts.tile([128, 2, E], fp8)
            maskb_sb = consts.tile([128, 128], bf16)
            mask8_sb = consts.tile([128, 128], fp8)
            bq_sb = consts.tile([128, 2], f32)

            # All DMAs use flat 2D APs on both sides: one contiguous
            # descriptor per partition (>=512B elements avoid the 2x
            # small-transfer penalty; fewer descriptors saturate the bus).
            def load_xtb(quarter, eng=None):
                ks = slice(quarter * 2, quarter * 2 + 2)
                (eng or nc.sync).dma_start(
                    out=xtb_sb[:, ks].rearrange("p k c -> p (k c)"),
                    in_=xtb[:, ks.start * 512 : ks.stop * 512],
                )

            def load_w(dst, src, g, eng=None):
                # mt/g-major: one contiguous 2KB transfer unblocks the whole
                # half projection chain
                (eng or nc.sync).dma_start(
                    out=dst[:, g].rearrange("p k c -> p (k c)"),
                    in_=src[:, g * KT * 128 : (g + 1) * KT * 128],
                )

            def load_xt8(qx, eng=None):  # chunk index 0..3
                (eng or nc.sync).dma_start(
                    out=xt8_sb[:, qx].rearrange("p k c -> p (k c)"),
                    in_=xt8[:, qx * KT * 512 : (qx + 1) * KT * 512],
                )

            # DMA issue order: tiny consts, att0's projection gate
            # (weights early so both g-chains run as x quarters land), wvb
            # before the fp8 gate (att0's PV needs it at ~8us), then att1's
            # fp8 gate, then the tail. x tensors ride a second queue.
            nc.sync.dma_start(out=bq_sb, in_=bq)
            nc.sync.dma_start(out=maskb_sb, in_=maskb)
            load_w(wqb_sb, wqb, 0)
            load_xtb(0, eng=nc.scalar)
            load_xtb(1, eng=nc.scalar)
            load_w(wkb_sb, wkb, 0)
            load_w(wqb_sb, wqb, 1)
            load_w(wkb_sb, wkb, 1)
            load_xtb(2, eng=nc.scalar)
            load_xtb(3, eng=nc.scalar)
            nc.sync.dma_start(
                out=wvb_sb.rearrange("p k c -> p (k c)"), in_=wvb
            )
            nc.sync.dma_start(
                out=wq8_sb.rearrange("p a i c -> p (a i c)"), in_=wq8
            )
            nc.sync.dma_start(
                out=wk8_sb.rearrange("p a i c -> p (a i c)"), in_=wk8
            )
            load_xt8(0, eng=nc.scalar)
            nc.sync.dma_start(out=mask8_sb, in_=mask8)
            nc.sync.dma_start(
                out=wv8_sb.rearrange("p a i c -> p (a i c)"), in_=wv8
            )
            nc.gpsimd.load_library(library_config.attn)
            load_xt8(1, eng=nc.scalar)
            nc.sync.dma_start(out=wob_sb.rearrange("p g c -> p (g c)"), in_=wob)
            nc.sync.dma_start(out=wo8_sb.rearrange("p g c -> p (g c)"), in_=wo8)
            load_xt8(2, eng=nc.scalar)

            # force the Exp activation-table load off the critical path
            warm = small.tile([1, 8], f32, tag="warm", name="warm")
            nc.vector.memset(warm, 0.0)
            warm2 = small.tile([1, 8], f32, tag="warm", name="warm2")
            nc.scalar.activation(warm2, warm, Exp)
            # exp bias tile for the fp8 path
            sh_sb = consts.tile([128, 1], f32)
            nc.vector.memset(sh_sb, -SHIFT)

            # PE p-state warm-up: ~3us of continuous dummy matmuls while the
            # first DMAs land, so the real projections start at full clock
            wu = consts.tile([128, 512], bf16)
            nc.vector.memset(wu, 0.0)
            wups = ps_pj.tile([128, 512], f32, tag="pj", name="warmup")
            for i in range(8):
                nc.tensor.matmul(
                    wups, lhsT=wu[:, 0:128], rhs=wu,
                    start=(i == 0), stop=(i == 7),
                )


            # ---- persistent activations ----
            qtb_sb = consts.tile([128, 2, 512], bf16)   # chunk-0 Q, g-split
            ktb_sb = consts.tile([128, 2, 512], bf16)   # chunk-0 K, g-split
            qt8_sb = consts.tile([128, 2, S], fp8)      # g-split fp8 Q
            kt8_sb = consts.tile([128, 2, S], fp8)      # g-split fp8 K
            vb_sb = consts.tile([128, 4, VBW], bf16)    # chunk-0 V + ones
            v8_sb = consts.tile([128, NQT, V8W], fp8)   # fp8 V + ones
            nc.vector.memset(
                vb_sb.rearrange("p rt (h c) -> p rt h c", h=HPC)[:, :, :, D : D + 1],
                1.0,
            )
            nc.vector.memset(
                v8_sb.rearrange("p rt (h c) -> p rt h c", h=HPC)[:, :, :, D : D + 1],
                1.0,
            )
            atb_sb = consts.tile([128, 2, 512], bf16)   # chunk-0 attout^T
            at8_sb = consts.tile([128, 2, S], fp8)      # fp8 attout^T
            y_sb = consts.tile([128, NQT, E], bf16)

            # ---- projection fillers ----
            def projb_qk():
                """bf16 chunk-0 Q,K (gates the first exp); K bf16 copies go
                on ACT, which is idle during startup."""
                fs = []
                for g in range(2):
                    for w_sb, kind in ((wqb_sb, "q"), (wkb_sb, "k")):
                        box = {}

                        def h1(w_sb=w_sb, g=g, kind=kind, box=box):
                            ps = ps_pj.tile([128, 512], f32, tag="pj",
                                            name=f"pjb_{kind}{g}")
                            box["ps"] = ps
                            for kt in range(4):
                                nc.tensor.matmul(
                                    ps, lhsT=w_sb[:, g, kt, :], rhs=xtb_sb[:, kt],
                                    start=(kt == 0), stop=False,
                                )

                        def h2(w_sb=w_sb, g=g, kind=kind, box=box):
                            ps = box["ps"]
                            for kt in range(4, 8):
                                nc.tensor.matmul(
                                    ps, lhsT=w_sb[:, g, kt, :], rhs=xtb_sb[:, kt],
                                    start=False, stop=(kt == 7),
                                )
                            if kind == "q":
                                nc.vector.tensor_scalar_add(
                                    qtb_sb[:, g], ps, bq_sb[:, g : g + 1]
                                )
                            else:
                                nc.scalar.copy(ktb_sb[:, g], ps)
                                nc.vector.tensor_copy(kt8_sb[:, g, 0:512], ps)

                        fs += [h1, h2]
                return fs

            def projb_v():
                """bf16 chunk-0 V (runs as attention(0) filler; PV lags
                scores so vb arrives in time)."""
                fs = []
                for rl in range(4):
                    box = {}

                    def v1(rl=rl, box=box):
                        ps = ps_pj.tile([128, 512], f32, tag="pj", name=f"pjb_v{rl}")
                        box["ps"] = ps
                        for kt in range(4):
                            nc.tensor.matmul(
                                ps[:, 0:HD],
                                lhsT=xtb_sb[:, kt, rl * 128 : rl * 128 + 128],
                                rhs=wvb_sb[:, kt],
                                start=(kt == 0), stop=False,
                            )

                    def v2(rl=rl, box=box):
                        ps = box["ps"]
                        for kt in range(4, 8):
                            nc.tensor.matmul(
                                ps[:, 0:HD],
                                lhsT=xtb_sb[:, kt, rl * 128 : rl * 128 + 128],
                                rhs=wvb_sb[:, kt],
                                start=False, stop=(kt == 7),
                            )
                        psh = ps[:, 0:HD].rearrange("p (h c) -> p h c", h=HPC)
                        nc.vector.tensor_copy(
                            vb_sb[:, rl].rearrange("p (h c) -> p h c", h=HPC)[
                                :, :, 0:D
                            ],
                            psh,
                        )
                        nc.vector.tensor_copy(
                            v8_sb[:, rl].rearrange("p (h c) -> p h c", h=HPC)[
                                :, :, 0:D
                            ],
                            psh,
                        )

                    fs += [v1, v2]
                return fs

            def proj8_fillers(qc):
                """fp8 DR projections for chunk qc in 1..3."""
                fs = []
                for w_sb, kind in ((wq8_sb, "q"), (wk8_sb, "k")):
                    for g in range(2):

                        def f(w_sb=w_sb, g=g, kind=kind, qc=qc):
                            ps = ps_pj.tile([128, 512], f32, tag="pj",
                                            name=f"pj8_{kind}{qc}{g}")
                            for t in range(4):
                                nc.tensor.matmul(
                                    ps,
                                    lhsT=w_sb[:, g * 4 + t],
                                    rhs=xt8_sb[:, qc - 1, 2 * t : 2 * t + 2],
                                    start=(t == 0), stop=(t == 3),
                                    perf_mode=DR,
                                )
                            dst = qt8_sb if kind == "q" else kt8_sb
                            if kind == "q":
                                nc.vector.tensor_scalar_add(
                                    dst[:, g, qc * 512 : (qc + 1) * 512],
                                    ps, bq_sb[:, g : g + 1],
                                )
                            else:
                                nc.vector.tensor_copy(
                                    dst[:, g, qc * 512 : (qc + 1) * 512], ps
                                )

                        fs.append(f)
                for rl in range(4):

                    def fv(rl=rl, qc=qc):
                        rt = qc * 4 + rl
                        ps = ps_pj.tile([128, 512], f32, tag="pj", name=f"pj8_v{rt}")
                        for t in range(4):
                            nc.tensor.matmul(
                                ps[:, 0:HD],
                                lhsT=xt8_sb[:, qc - 1, 2 * t : 2 * t + 2,
                                            rl * 128 : rl * 128 + 128],
                                rhs=wv8_sb[:, t],
                                start=(t == 0), stop=(t == 3),
                                perf_mode=DR,
                            )
                        nc.vector.tensor_copy(
                            v8_sb[:, rt].rearrange("p (h c) -> p h c", h=HPC)[
                                :, :, 0:D
                            ],
                            ps[:, 0:HD].rearrange("p (h c) -> p h c", h=HPC),
                        )

                    fs.append(fv)
                return fs

            # ---- out-projection fillers ----
            def outproj_fillers(qc, use_act=False):
                fs = []
                for qtl in range(4):
                    qt = qc * 4 + qtl
                    for nh in range(2):

                        def f(qc=qc, qt=qt, qtl=qtl, nh=nh):
                            ps = ps_pj.tile([128, 512], f32, tag="pj",
                                            name=f"pj_y{qt}{nh}")
                            if qc == 0:
                                for kt2 in range(2):
                                    nc.tensor.matmul(
                                        ps,
                                        lhsT=atb_sb[:, kt2,
                                                    qtl * 128 : qtl * 128 + 128],
                                        rhs=wob_sb[:, kt2, nh * 512 : nh * 512 + 512],
                                        start=(kt2 == 0), stop=(kt2 == 1),
                                    )
                            else:
                                nc.tensor.matmul(
                                    ps,
                                    lhsT=at8_sb[:, :, qt * 128 : qt * 128 + 128],
                                    rhs=wo8_sb[:, :, nh * 512 : nh * 512 + 512],
                                    start=True, stop=True,
                                    perf_mode=DR,
                                )
                            dst = y_sb[:, qt, nh * 512 : nh * 512 + 512]
                            if use_act and nh == 0:
                                nc.scalar.copy(dst, ps)
                            else:
                                nc.vector.tensor_copy(dst, ps)
                            if nh == 1:
                                nc.sync.dma_start(
                                    out=y[:, qt * E : (qt + 1) * E],
                                    in_=y_sb[:, qt],
                                )

                        fs.append(f)
                return fs

            # ---- normalize one head-pair: attout = acc[0:64] / acc[64] ----
            def normalize(qc, mt, acc, final=False):
                dst = atb_sb if qc == 0 else at8_sb
                col0 = 0 if qc == 0 else qc * 512
                rc = small.tile([1, 2, 512], f32, tag="rc", name="rc")
                if not final:
                    nc.vector.reciprocal(out=rc[:, 0, :], in_=acc[0][64:65, :])
                    nc.vector.reciprocal(out=rc[:, 1, :], in_=acc[1][64:65, :])
                    for hp in range(2):
                        bc = small.tile([64, 512], f32, tag="bc", name=f"bc{hp}")
                        nc.gpsimd.partition_broadcast(
                            out_ap=bc, in_ap=rc[:, hp, :]
                        )
                        pb = hp * 64
                        nc.vector.tensor_mul(
                            dst[pb : pb + 64, mt, col0 : col0 + 512],
                            acc[hp][0:64, :], bc,
                        )
                else:
                    # final chunk: pipeline the whole normalize in 128-col
                    # pieces so the epilogue matmuls start ASAP
                    bcs = [small.tile([64, 512], f32, tag="bc", name=f"bc{hp}")
                           for hp in range(2)]
                    for qtl in range(4):
                        cols = slice(qtl * 128, qtl * 128 + 128)
                        for hp in range(2):
                            nc.vector.reciprocal(
                                out=rc[:, hp, cols], in_=acc[hp][64:65, cols]
                            )
                            nc.gpsimd.partition_broadcast(
                                out_ap=bcs[hp][:, cols], in_ap=rc[:, hp, cols]
                            )
                            pb = hp * 64
                            nc.vector.tensor_mul(
                                dst[pb : pb + 64, mt,
                                    col0 + qtl * 128 : col0 + qtl * 128 + 128],
                                acc[hp][0:64, cols], bcs[hp][:, cols],
                            )

            # ---- qc=0 attention: bf16 path ----
            def attention_bf16(fillers):
                ti = fi = 0
                ntiles = 2 * 4
                for mt in range(2):
                    acc = [
                        ps_acc.tile([128, 512], f32, tag="acc", name=f"accb{mt}{hp}")
                        for hp in range(2)
                    ]

                    def pv(kt, ex, off):
                        for hp in range(2):
                            h = 2 * mt + hp
                            nc.tensor.matmul(
                                acc[hp][0:65, off:512],
                                lhsT=vb_sb[:, kt, h * 65 : h * 65 + 65],
                                rhs=ex[:, hp, off:512],
                                start=(kt == 0), stop=(kt == 3),
                            )

                    pend = []
                    for kt in range(4):
                        off = 128 * kt if kt > 0 else 0
                        ps = ps_sc.tile([128, 2, 512], f32, tag="sc",
                                        name=f"scb{mt}{kt}")
                        for hp in range(2):
                            h = 2 * mt + hp
                            p0 = h * 32
                            for g in range(2):
                                nc.tensor.matmul(
                                    ps[:, hp, off:512],
                                    lhsT=ktb_sb[p0 : p0 + 32, g,
                                                kt * 128 : kt * 128 + 128],
                                    rhs=qtb_sb[p0 : p0 + 32, g, off:512],
                                    start=(g == 0), stop=(g == 1),
                                    tile_position=(p0, 0),
                                )
                        ex = exps.tile([128, 2, 512], bf16, tag="exb",
                                       name=f"exb{kt}")
                        nc.scalar.activation(
                            ex[:, :, off:512], ps[:, :, off:512], Exp, scale=0.125
                        )
                        # diagonal band mask (every qc0 tile is diagonal)
                        for hp in range(2):
                            nc.vector.tensor_mul(
                                ex[:, hp, off : off + 128],
                                ex[:, hp, off : off + 128],
                                maskb_sb,
                            )
                        ti += 1
                        want = min(len(fillers),
                                   ti * 3 * len(fillers) // (2 * ntiles))
                        while fi < want:
                            fillers[fi]()
                            fi += 1
                        if len(pend) == 3:
                            pv(*pend.pop(0))
                        pend.append((kt, ex, off))
                    for p in pend:
                        pv(*p)
                    normalize(0, mt, acc)
                while fi < len(fillers):
                    fillers[fi]()
                    fi += 1

            # ---- qc>=1 attention: fp8 DR path ----
            def attention_fp8(qc, fillers, pre_tail=None):
                nkt = 4 * (qc + 1)
                npair = nkt // 2
                ntiles = 2 * nkt
                ti = fi = 0
                for mt in range(2):
                    acc = [
                        ps_acc.tile([128, 512], f32, tag="acc",
                                    name=f"acc8{qc}{mt}{hp}")
                        for hp in range(2)
                    ]

                    def pv(m, ex, off0, off1):
                        # DR over the pair intersection [off1:512]; the
                        # earlier tile's extra band [off0:off1) as a plain
                        # fp8 matmul
                        for hp in range(2):
                            h = 2 * mt + hp
                            if off1 > off0:
                                nc.tensor.matmul(
                                    acc[hp][0:65, off0:off1],
                                    lhsT=v8_sb[:, 2 * m, h * 80 : h * 80 + 65],
                                    rhs=ex[:, hp, 0, off0:off1],
                                    start=False, stop=False,
                                )
                            nc.tensor.matmul(
                                acc[hp][0:65, off1:512],
                                lhsT=v8_sb[:, 2 * m : 2 * m + 2,
                                           h * 80 : h * 80 + 65],
                                rhs=ex[:, hp, :, off1:512],
                                start=(m == 0), stop=(m == npair - 1),
                                perf_mode=DR,
                            )

                    pend = []
                    for m in range(npair):
                        ex = exps.tile([128, 2, 2, 512], fp8, tag="ex8",
                                       name=f"ex8{m % 3}")
                        offs = []
                        for sl in range(2):
                            kt = 2 * m + sl
                            t = kt - 4 * qc
                            off = 128 * t if t > 0 else 0
                            offs.append(off)
                            ps = ps_sc.tile([128, 2, 512], f32, tag="sc",
                                            name=f"sc8{qc}{mt}{kt}")
                            for hp in range(2):
                                h = 2 * mt + hp
                                p0 = h * 32
                                nc.tensor.matmul(
                                    ps[:, hp, off:512],
                                    lhsT=kt8_sb[p0 : p0 + 32, :,
                                                kt * 128 : kt * 128 + 128],
                                    rhs=qt8_sb[p0 : p0 + 32, :,
                                               qc * 512 + off : (qc + 1) * 512],
                                    start=True, stop=True,
                                    perf_mode=DR,
                                    tile_position=(p0, 0),
                                )
                            nc.scalar.activation(
                                ex[:, :, sl, off:512], ps[:, :, off:512],
                                Exp, scale=0.125, bias=sh_sb,
                            )
                            if t >= 0:
                                # final stretch: DVE masks keep the critical
                                # exp->mask->PV chain off the Pool queue
                                meng = (nc.vector
                                        if qc == QC - 1 and mt == 1
                                        else nc.gpsimd)
                                for hp in range(2):
                                    meng.tensor_mul(
                                        ex[:, hp, sl, off : off + 128],
                                        ex[:, hp, sl, off : off + 128],
                                        mask8_sb,
                                    )
                            ti += 1
                            want = min(len(fillers),
                                   ti * 3 * len(fillers) // (2 * ntiles))
                            while fi < want:
                                fillers[fi]()
                                fi += 1
                        if len(pend) == 4:
                            pv(*pend.pop(0))
                        pend.append((m, ex, offs[0], offs[1]))
                    for p in pend:
                        pv(*p)
                    if mt == 1 and pre_tail is not None:
                        pre_tail()
                    normalize(qc, mt, acc, final=(qc == QC - 1 and mt == 1))
                while fi < len(fillers):
                    fillers[fi]()
                    fi += 1

            # ---- schedule ----
            for f in projb_qk():
                f()
            attention_bf16(projb_v() + proj8_fillers(1))
            attention_fp8(1, proj8_fillers(2) + outproj_fillers(0))
            attention_fp8(2, proj8_fillers(3) + outproj_fillers(1))

            # split epilogue: gr0 halves of out-proj(3) only need at8 gr 0
            # (mt=0, normalized before mt=1 runs), so they are issued right
            # after the mt=1 PV drain to keep PE busy through the final
            # normalize; gr1 halves + copies follow.
            ep_groups = [(qt, nh) for qt in range(12, 16) for nh in range(2)]
            ep_slots = {}

            def ep_phase_a():
                slots = [
                    ps_pj.tile([128, 512], f32, tag="pj", name="ep_pj0"),
                    ps_pj.tile([128, 512], f32, tag="pj", name="ep_pj1"),
                ]
                for i in range(2):
                    sc = ps_sc.tile([128, 2, 512], f32, tag="sc", name=f"ep_sc{i}")
                    slots += [sc[:, 0, :], sc[:, 1, :]]
                for i, (qt, nh) in enumerate(ep_groups[:6]):
                    ep_slots[(qt, nh)] = slots[i]
                    nc.tensor.matmul(
                        slots[i],
                        lhsT=at8_sb[:, 0, qt * 128 : qt * 128 + 128],
                        rhs=wo8_sb[:, 0, nh * 512 : nh * 512 + 512],
                        start=True, stop=False,
                    )

            attention_fp8(3, outproj_fillers(2), pre_tail=ep_phase_a)

            for i, (qt, nh) in enumerate(ep_groups):
                if (qt, nh) in ep_slots:
                    ps = ep_slots[(qt, nh)]
                    nc.tensor.matmul(
                        ps,
                        lhsT=at8_sb[:, 1, qt * 128 : qt * 128 + 128],
                        rhs=wo8_sb[:, 1, nh * 512 : nh * 512 + 512],
                        start=False, stop=True,
                    )
                else:
                    ps = ps_pj.tile([128, 512], f32, tag="pj", name=f"ep_y{qt}{nh}")
                    nc.tensor.matmul(
                        ps,
                        lhsT=at8_sb[:, :, qt * 128 : qt * 128 + 128],
                        rhs=wo8_sb[:, :, nh * 512 : nh * 512 + 512],
                        start=True, stop=True, perf_mode=DR,
                    )
                dst = y_sb[:, qt, nh * 512 : nh * 512 + 512]
                nc.scalar.copy(dst, ps)
                nc.sync.dma_start(
                    out=y[:, qt * E + nh * 512 : qt * E + nh * 512 + 512],
                    in_=dst,
                )

    nc.compile()
    return nc


def _get_program():
    global _prog
    if _prog is None:
        _prog = _build_program()
    return _prog


def _masks():
    import ml_dtypes

    k = np.arange(128)[:, None]
    q = np.arange(128)[None, :]
    m = np.ascontiguousarray(q >= k)
    return m.astype(ml_dtypes.bfloat16), m.astype(ml_dtypes.float8_e4m3)


def _gsplit_cols(bias=False):
    """Column permutation for the g-split feature packing.

    feature index f in [0,256): g = f//128, head = (f%128)//32,
    d = 32*g + f%32 -> source col = head*64 + d.
    """
    f = np.arange(256)
    g, r = f // 128, f % 128
    return (r // 32) * 64 + g * 32 + (f % 32)


def _core_inputs(x, Wq, bq, Wk, Wv, Wo, maskb, mask8, c):
    import ml_dtypes

    nbf = ml_dtypes.bfloat16
    nf8 = ml_dtypes.float8_e4m3
    b, g = divmod(c, 4)
    sl = slice(g * HD, (g + 1) * HD)
    xT = np.ascontiguousarray(x[b].T)  # [E, S]
    # [128, kt, cols] packs
    xTr = xT.reshape(KT, 128, S)
    xtb_p = np.ascontiguousarray(
        xTr[:, :, 0:512].transpose(1, 0, 2).reshape(128, KT * 512)
    )
    xt8_p = np.ascontiguousarray(
        xTr[:, :, 512:2048].reshape(KT, 128, 3, 512)
        .transpose(1, 2, 0, 3).reshape(128, 3 * KT * 512)
    )
    cols = _gsplit_cols()
    Wqs, Wks = Wq[:, sl][:, cols], Wk[:, sl][:, cols]  # [1024, 2*128] g-split
    # bf16, g-major g-split: [128, g, kt, 128]
    def packb(W):
        return np.ascontiguousarray(
            W.reshape(KT, 128, 2, 128).transpose(1, 2, 0, 3).reshape(128, -1)
        )
    # fp8: [128, (g,t)=8, i, 128]: a = g*4+t, rows (2t+i)*128
    def pack8(W):
        Wr = W.reshape(4, 2, 128, 2, 128)  # [t, i, p, g, f]
        return np.ascontiguousarray(
            Wr.transpose(2, 3, 0, 1, 4).reshape(128, 2 * 4 * 2 * 128)
        )
    wvr = Wv[:, sl].reshape(4, 2, 128, HD)  # [t, i, p, c]
    wv8_p = np.ascontiguousarray(wvr.transpose(2, 0, 1, 3).reshape(128, -1))
    wvb_p = np.ascontiguousarray(
        Wv[:, sl].reshape(KT, 128, HD).transpose(1, 0, 2).reshape(128, -1)
    )
    wo_p = np.ascontiguousarray(
        Wo[sl, :].reshape(2, 128, E).transpose(1, 0, 2).reshape(128, -1)
    )
    bqg = np.ascontiguousarray(bq[sl][_gsplit_cols()].reshape(2, 128).T)
    return {
        "xtb": xtb_p.astype(nbf),
        "xt8": xt8_p.astype(nf8),
        "wqb": packb(Wqs).astype(nbf),
        "wkb": packb(Wks).astype(nbf),
        "wq8": pack8(Wqs).astype(nf8),
        "wk8": pack8(Wks).astype(nf8),
        "wvb": wvb_p.astype(nbf),
        "wv8": wv8_p.astype(nf8),
        "wob": wo_p.astype(nbf),
        "wo8": wo_p.astype(nf8),
        "bqc": bqg.astype(np.float32),
        "maskb": maskb,
        "mask8": mask8,
    }


def _unpack_y(y_p):
    """[128, NQT*E] -> [S, E]"""
    return y_p.reshape(128, NQT, E).transpose(1, 0, 2).reshape(S, E)


def kernel(x, Wq, bq, Wk, bk, Wv, bv, Wo, bo, **_run_kwargs):
    from concourse.bass_utils import run_bass_kernel_spmd

    x = np.asarray(x, dtype=np.float32)
    Wq, bq = np.asarray(Wq, np.float32), np.asarray(bq, np.float32)
    Wk, bk = np.asarray(Wk, np.float32), np.asarray(bk, np.float32)
    Wv, bv = np.asarray(Wv, np.float32), np.asarray(bv, np.float32)
    Wo, bo = np.asarray(Wo, np.float32), np.asarray(bo, np.float32)

    nc = _get_program()
    maskb, mask8 = _masks()
    in_maps = [
        _core_inputs(x, Wq, bq, Wk, Wv, Wo, maskb, mask8, c) for c in range(NCORES)
    ]
    res = run_bass_kernel_spmd(nc, in_maps, list(range(NCORES)), **_run_kwargs)
    global LAST_RESULTS
    LAST_RESULTS = res
    parts = [_unpack_y(res.results[c]["y"].astype(np.float32)) for c in range(NCORES)]
    # bias identities: bk drops out of softmax; bv contributes bv @ Wo
    bias = bo + bv @ Wo
    out = np.empty((B, S, E), np.float32)
    for b in range(B):
        out[b] = parts[4 * b] + parts[4 * b + 1] + parts[4 * b + 2] + parts[4 * b + 3]
        out[b] += bias
    return out


# revision 38
# speedup vs baseline: 1.0029x; 1.0029x over previous
"""Causal self-attention (B=2, S=2048, E=1024, H=16, D=64) on 8 TRN2 cores.

Sharding: core c = (batch b = c//4, head-group g = c%4) owns batch b and
heads 4g..4g+3 (a 256-wide slice of the QKV projections / Wo rows).
Each core computes its partial out-projection y_c = attout_c @ Wo_c; the
host sums the 4 partials per batch and adds the folded bias (bk drops out
of softmax; bv contributes bv @ Wo since softmax weights sum to 1).

Row-dependent precision (validated: end-to-end rel err ~8e-3 vs 2e-2 gate):
  - queries 0-511 (qc=0): bf16 pipeline. Early rows have concentrated
    softmax and O(1)-magnitude attout, so they need bf16.
  - queries 512+ (qc=1..3): fp8e4m3 pipeline with DoubleRow (DR) matmuls.
    attout magnitude ~1.65/sqrt(L) for context length L, so fp8's ~4%
    relative noise stays far below the absolute error budget.
  DR matmul: lhsT [K,2,M] fp8, rhs [K,2,N] fp8 -> out[M,N] = sum_g
  lhsT[:,g].T @ rhs[:,g], at 0.5 PE cycles/output-column (4x bf16
  throughput per contraction element). DR free-dim group strides must be
  16-byte aligned -> V head slots padded to 80 cols.

Layouts:
  Q^T/K^T [128, 2, S]: partition p = head(p//32)*32 + d%32, free dim g =
  d//32 (the 32+32 d-split lets scores contract d=64 as DR [32,2]).
  V [128, rt, 320]: natural keys-on-partitions; per head h cols h*80..+63
  are V, col h*80+64 is ones (PV row 64 = softmax denominator).
  attout^T [128, 2, S]: partitions (h%2)*64+d, group h//2 -- matches the
  Wo row packing so out-proj is a single DR matmul per (qt, nh).

exp runs on ACT (the end-to-end bottleneck: ~58us of causal-area exp), in
[128, 2hp, 512-off] tiles; fp8-path exp folds scale 1/8 and bias -1.5
(softmax-invariant shift that keeps exp below e4m3's 240 max). The causal
mask multiply runs on GPSIMD for the fp8 path (SBUF-only engine), DVE for
the bf16 path. PE idle during exp is filled with the next chunk's
projections / previous chunks' out-projections (baseline's filler
interleave, engine queues are in-order).
"""

import numpy as np

B, S, E, H = 2, 2048, 1024, 16
D = E // H          # 64
NCORES = 8
HPC = 4             # heads per core
HD = HPC * D        # 256 cols per core
KT = E // 128       # 8 contraction tiles
QC = S // 512       # 4 query chunks
NQT = S // 128      # 16 row tiles
V8W = HPC * 80      # 320: fp8 V with 80-wide head slots (16B-aligned)
VBW = HPC * (D + 1)  # 260: bf16 V with ones column per head
SHIFT = 1.5         # exp bias: exp(s/8 - SHIFT), cancels in softmax

_prog = None
LAST_RESULTS = None


def _build_program():
    import concourse.mybir as mybir
    import concourse.tile as tile
    from concourse import bacc, library_config

    f32 = mybir.dt.float32
    bf16 = mybir.dt.bfloat16
    fp8 = mybir.dt.float8e4
    Exp = mybir.ActivationFunctionType.Exp
    DR = mybir.MatmulPerfMode.DoubleRow

    nc = bacc.Bacc(trn_type="TRN2", target_bir_lowering=False, debug=False)

    xtb = nc.dram_tensor("xtb", [128, KT * 512], bf16, kind="ExternalInput").ap()
    xt8 = nc.dram_tensor("xt8", [128, 3 * KT * 512], fp8, kind="ExternalInput").ap()
    wqb = nc.dram_tensor("wqb", [128, KT * 2 * 128], bf16, kind="ExternalInput").ap()
    wkb = nc.dram_tensor("wkb", [128, KT * 2 * 128], bf16, kind="ExternalInput").ap()
    wq8 = nc.dram_tensor("wq8", [128, 8 * 2 * 128], fp8, kind="ExternalInput").ap()
    wk8 = nc.dram_tensor("wk8", [128, 8 * 2 * 128], fp8, kind="ExternalInput").ap()
    wvb = nc.dram_tensor("wvb", [128, KT * 256], bf16, kind="ExternalInput").ap()
    wv8 = nc.dram_tensor("wv8", [128, 4 * 2 * 256], fp8, kind="ExternalInput").ap()
    wob = nc.dram_tensor("wob", [128, 2 * E], bf16, kind="ExternalInput").ap()
    wo8 = nc.dram_tensor("wo8", [128, 2 * E], fp8, kind="ExternalInput").ap()
    bq = nc.dram_tensor("bqc", [128, 2], f32, kind="ExternalInput").ap()
    # lower-triangular band mask (valid iff q_local >= k), bf16 + fp8 copies
    maskb = nc.dram_tensor("maskb", [128, 128], bf16, kind="ExternalInput").ap()
    mask8 = nc.dram_tensor("mask8", [128, 128], fp8, kind="ExternalInput").ap()
    y = nc.dram_tensor("y", [128, NQT * E], bf16, kind="ExternalOutput").ap()

    with tile.TileContext(nc) as tc:
        with (
            tc.tile_pool(name="consts", bufs=1) as consts,
            tc.tile_pool(name="exps", bufs=8) as exps,
            tc.tile_pool(name="small", bufs=4) as small,
            tc.tile_pool(name="ps_sc", bufs=2, space="PSUM") as ps_sc,
            tc.tile_pool(name="ps_pj", bufs=2, space="PSUM") as ps_pj,
            tc.tile_pool(name="ps_acc", bufs=2, space="PSUM") as ps_acc,
        ):
            # ---- SBUF constants; DMA issue order = need order ----
            xtb_sb = consts.tile([128, KT, 512], bf16)
            xt8_sb = consts.tile([128, 3, KT, 512], fp8)
            wqb_sb = consts.tile([128, 2, KT, 128], bf16)
            wkb_sb = consts.tile([128, 2, KT, 128], bf16)
            wq8_sb = consts.tile([128, 8, 2, 128], fp8)
            wk8_sb = consts.tile([128, 8, 2, 128], fp8)
            wvb_sb = consts.tile([128, KT, 256], bf16)
            wv8_sb = consts.tile([128, 4, 2, 256], fp8)
            wob_sb = consts.tile([128, 2, E], bf16)
            wo8_sb = consts.tile([128, 2, E], fp8)
            maskb_sb = consts.tile([128, 128], bf16)
            mask8_sb = consts.tile([128, 128], fp8)
            bq_sb = consts.tile([128, 2], f32)

            # All DMAs use flat 2D APs on both sides: one contiguous
            # descriptor per partition (>=512B elements avoid the 2x
            # small-transfer penalty; fewer descriptors saturate the bus).
            def load_xtb(quarter, eng=None):
                ks = slice(quarter * 2, quarter * 2 + 2)
                (eng or nc.sync).dma_start(
                    out=xtb_sb[:, ks].rearrange("p k c -> p (k c)"),
                    in_=xtb[:, ks.start * 512 : ks.stop * 512],
                )

            def load_w(dst, src, g, eng=None):
                # mt/g-major: one contiguous 2KB transfer unblocks the whole
                # half projection chain
                (eng or nc.sync).dma_start(
                    out=dst[:, g].rearrange("p k c -> p (k c)"),
                    in_=src[:, g * KT * 128 : (g + 1) * KT * 128],
                )

            def load_xt8(qx, eng=None):  # chunk index 0..3
                (eng or nc.sync).dma_start(
                    out=xt8_sb[:, qx].rearrange("p k c -> p (k c)"),
                    in_=xt8[:, qx * KT * 512 : (qx + 1) * KT * 512],
                )

            # DMA issue order: tiny consts, att0's projection gate
            # (weights early so both g-chains run as x quarters land), wvb
            # before the fp8 gate (att0's PV needs it at ~8us), then att1's
            # fp8 gate, then the tail. x tensors ride a second queue.
            nc.sync.dma_start(out=bq_sb, in_=bq)
            nc.sync.dma_start(out=maskb_sb, in_=maskb)
            load_w(wqb_sb, wqb, 0)
            load_xtb(0, eng=nc.scalar)
            load_xtb(1, eng=nc.scalar)
            load_w(wkb_sb, wkb, 0)
            load_w(wqb_sb, wqb, 1)
            load_w(wkb_sb, wkb, 1)
            load_xtb(2, eng=nc.scalar)
            load_xtb(3, eng=nc.scalar)
            nc.sync.dma_start(
                out=wvb_sb.rearrange("p k c -> p (k c)"), in_=wvb
            )
            nc.sync.dma_start(
                out=wq8_sb.rearrange("p a i c -> p (a i c)"), in_=wq8
            )
            nc.sync.dma_start(
                out=wk8_sb.rearrange("p a i c -> p (a i c)"), in_=wk8
            )
            load_xt8(0, eng=nc.scalar)
            nc.sync.dma_start(out=mask8_sb, in_=mask8)
            nc.sync.dma_start(
                out=wv8_sb.rearrange("p a i c -> p (a i c)"), in_=wv8
            )
            nc.gpsimd.load_library(library_config.attn)
            load_xt8(1, eng=nc.scalar)
            nc.sync.dma_start(out=wob_sb.rearrange("p g c -> p (g c)"), in_=wob)
            nc.sync.dma_start(out=wo8_sb.rearrange("p g c -> p (g c)"), in_=wo8)
            load_xt8(2, eng=nc.scalar)

            # force the Exp activation-table load off the critical path
            warm = small.tile([1, 8], f32, tag="warm", name="warm")
            nc.vector.memset(warm, 0.0)
            warm2 = small.tile([1, 8], f32, tag="warm", name="warm2")
            nc.scalar.activation(warm2, warm, Exp)
            # exp bias tile for the fp8 path
            sh_sb = consts.tile([128, 1], f32)
            nc.vector.memset(sh_sb, -SHIFT)

            # PE p-state warm-up: ~3us of continuous dummy matmuls while the
            # first DMAs land, so the real projections start at full clock
            wu = consts.tile([128, 512], bf16)
            nc.vector.memset(wu, 0.0)
            wups = ps_pj.tile([128, 512], f32, tag="pj", name="warmup")
            for i in range(8):
                nc.tensor.matmul(
                    wups, lhsT=wu[:, 0:128], rhs=wu,
                    start=(i == 0), stop=(i == 7),
                )


            # ---- persistent activations ----
            qtb_sb = consts.tile([128, 2, 512], bf16)   # chunk-0 Q, g-split
            ktb_sb = consts.tile([128, 2, 512], bf16)   # chunk-0 K, g-split
            qt8_sb = consts.tile([128, 2, S], fp8)      # g-split fp8 Q
            kt8_sb = consts.tile([128, 2, S], fp8)      # g-split fp8 K
            vb_sb = consts.tile([128, 4, VBW], bf16)    # chunk-0 V + ones
            v8_sb = consts.tile([128, NQT, V8W], fp8)   # fp8 V + ones
            nc.vector.memset(
                vb_sb.rearrange("p rt (h c) -> p rt h c", h=HPC)[:, :, :, D : D + 1],
                1.0,
            )
            nc.vector.memset(
                v8_sb.rearrange("p rt (h c) -> p rt h c", h=HPC)[:, :, :, D : D + 1],
                1.0,
            )
            atb_sb = consts.tile([128, 2, 512], bf16)   # chunk-0 attout^T
            at8_sb = consts.tile([128, 2, S], fp8)      # fp8 attout^T
            y_sb = consts.tile([128, NQT, E], bf16)

            # ---- projection fillers ----
            def projb_qk():
                """bf16 chunk-0 Q,K (gates the first exp); K bf16 copies go
                on ACT, which is idle during startup."""
                fs = []
                for g in range(2):
                    for w_sb, kind in ((wqb_sb, "q"), (wkb_sb, "k")):
                        box = {}

                        def h1(w_sb=w_sb, g=g, kind=kind, box=box):
                            ps = ps_pj.tile([128, 512], f32, tag="pj",
                                            name=f"pjb_{kind}{g}")
                            box["ps"] = ps
                            for kt in range(4):
                                nc.tensor.matmul(
                                    ps, lhsT=w_sb[:, g, kt, :], rhs=xtb_sb[:, kt],
                                    start=(kt == 0), stop=False,
                                )

                        def h2(w_sb=w_sb, g=g, kind=kind, box=box):
                            ps = box["ps"]
                            for kt in range(4, 8):
                                nc.tensor.matmul(
                                    ps, lhsT=w_sb[:, g, kt, :], rhs=xtb_sb[:, kt],
                                    start=False, stop=(kt == 7),
                                )
                            if kind == "q":
                                nc.vector.tensor_scalar_add(
                                    qtb_sb[:, g], ps, bq_sb[:, g : g + 1]
                                )
                            else:
                                nc.scalar.copy(ktb_sb[:, g], ps)
                                nc.vector.tensor_copy(kt8_sb[:, g, 0:512], ps)

                        fs += [h1, h2]
                return fs

            def projb_v():
                """bf16 chunk-0 V (runs as attention(0) filler; PV lags
                scores so vb arrives in time)."""
                fs = []
                for rl in range(4):
                    box = {}

                    def v1(rl=rl, box=box):
                        ps = ps_pj.tile([128, 512], f32, tag="pj", name=f"pjb_v{rl}")
                        box["ps"] = ps
                        for kt in range(4):
                            nc.tensor.matmul(
                                ps[:, 0:HD],
                                lhsT=xtb_sb[:, kt, rl * 128 : rl * 128 + 128],
                                rhs=wvb_sb[:, kt],
                                start=(kt == 0), stop=False,
                            )

                    def v2(rl=rl, box=box):
                        ps = box["ps"]
                        for kt in range(4, 8):
                            nc.tensor.matmul(
                                ps[:, 0:HD],
                                lhsT=xtb_sb[:, kt, rl * 128 : rl * 128 + 128],
                                rhs=wvb_sb[:, kt],
                                start=False, stop=(kt == 7),
                            )
                        psh = ps[:, 0:HD].rearrange("p (h c) -> p h c", h=HPC)
                        nc.vector.tensor_copy(
                            vb_sb[:, rl].rearrange("p (h c) -> p h c", h=HPC)[
                                :, :, 0:D
                            ],
                            psh,
                        )
                        nc.vector.tensor_copy(
                            v8_sb[:, rl].rearrange("p (h c) -> p h c", h=HPC)[
                                :, :, 0:D
                            ],
                            psh,
                        )

                    fs += [v1, v2]
                return fs

            def proj8_fillers(qc):
                """fp8 DR projections for chunk qc in 1..3."""
                fs = []
                for w_sb, kind in ((wq8_sb, "q"), (wk8_sb, "k")):
                    for g in range(2):

                        def f(w_sb=w_sb, g=g, kind=kind, qc=qc):
                            ps = ps_pj.tile([128, 512], f32, tag="pj",
                                            name=f"pj8_{kind}{qc}{g}")
                            for t in range(4):
                                nc.tensor.matmul(
                                    ps,
                                    lhsT=w_sb[:, g * 4 + t],
                                    rhs=xt8_sb[:, qc - 1, 2 * t : 2 * t + 2],
                                    start=(t == 0), stop=(t == 3),
                                    perf_mode=DR,
                                )
                            dst = qt8_sb if kind == "q" else kt8_sb
                            if kind == "q":
                                nc.vector.tensor_scalar_add(
                                    dst[:, g, qc * 512 : (qc + 1) * 512],
                                    ps, bq_sb[:, g : g + 1],
                                )
                            else:
                                nc.vector.tensor_copy(
                                    dst[:, g, qc * 512 : (qc + 1) * 512], ps
                                )

                        fs.append(f)
                for rl in range(4):

                    def fv(rl=rl, qc=qc):
                        rt = qc * 4 + rl
                        ps = ps_pj.tile([128, 512], f32, tag="pj", name=f"pj8_v{rt}")
                        for t in range(4):
                            nc.tensor.matmul(
                                ps[:, 0:HD],
                                lhsT=xt8_sb[:, qc - 1, 2 * t : 2 * t + 2,
                                            rl * 128 : rl * 128 + 128],
                                rhs=wv8_sb[:, t],
                                start=(t == 0), stop=(t == 3),
                                perf_mode=DR,
                            )
                        nc.vector.tensor_copy(
                            v8_sb[:, rt].rearrange("p (h c) -> p h c", h=HPC)[
                                :, :, 0:D
                            ],
                            ps[:, 0:HD].rearrange("p (h c) -> p h c", h=HPC),
                        )

                    fs.append(fv)
                return fs

            # ---- out-projection fillers ----
            def outproj_fillers(qc, use_act=False):
                fs = []
                for qtl in range(4):
                    qt = qc * 4 + qtl
                    for nh in range(2):

                        def f(qc=qc, qt=qt, qtl=qtl, nh=nh):
                            ps = ps_pj.tile([128, 512], f32, tag="pj",
                                            name=f"pj_y{qt}{nh}")
                            if qc == 0:
                                for kt2 in range(2):
                                    nc.tensor.matmul(
                                        ps,
                                        lhsT=atb_sb[:, kt2,
                                                    qtl * 128 : qtl * 128 + 128],
                                        rhs=wob_sb[:, kt2, nh * 512 : nh * 512 + 512],
                                        start=(kt2 == 0), stop=(kt2 == 1),
                                    )
                            else:
                                nc.tensor.matmul(
                                    ps,
                                    lhsT=at8_sb[:, :, qt * 128 : qt * 128 + 128],
                                    rhs=wo8_sb[:, :, nh * 512 : nh * 512 + 512],
                                    start=True, stop=True,
                                    perf_mode=DR,
                                )
                            dst = y_sb[:, qt, nh * 512 : nh * 512 + 512]
                            if use_act and nh == 0:
                                nc.scalar.copy(dst, ps)
                            else:
                                nc.vector.tensor_copy(dst, ps)
                            if nh == 1:
                                nc.sync.dma_start(
                                    out=y[:, qt * E : (qt + 1) * E],
                                    in_=y_sb[:, qt],
                                )

                        fs.append(f)
                return fs

            # ---- normalize one head-pair: attout = acc[0:64] / acc[64] ----
            def normalize(qc, mt, acc, final=False):
                dst = atb_sb if qc == 0 else at8_sb
                col0 = 0 if qc == 0 else qc * 512
                rc = small.tile([1, 2, 512], f32, tag="rc", name="rc")
                if not final:
                    nc.vector.reciprocal(out=rc[:, 0, :], in_=acc[0][64:65, :])
                    nc.vector.reciprocal(out=rc[:, 1, :], in_=acc[1][64:65, :])
                    for hp in range(2):
                        bc = small.tile([64, 512], f32, tag="bc", name=f"bc{hp}")
                        nc.gpsimd.partition_broadcast(
                            out_ap=bc, in_ap=rc[:, hp, :]
                        )
                        pb = hp * 64
                        nc.vector.tensor_mul(
                            dst[pb : pb + 64, mt, col0 : col0 + 512],
                            acc[hp][0:64, :], bc,
                        )
                else:
                    # final chunk: pipeline the whole normalize in 128-col
                    # pieces so the epilogue matmuls start ASAP
                    bcs = [small.tile([64, 512], f32, tag="bc", name=f"bc{hp}")
                           for hp in range(2)]
                    for qtl in range(4):
                        cols = slice(qtl * 128, qtl * 128 + 128)
                        for hp in range(2):
                            nc.vector.reciprocal(
                                out=rc[:, hp, cols], in_=acc[hp][64:65, cols]
                            )
                            nc.gpsimd.partition_broadcast(
                                out_ap=bcs[hp][:, cols], in_ap=rc[:, hp, cols]
                            )
                            pb = hp * 64
                            nc.vector.tensor_mul(
                                dst[pb : pb + 64, mt,
                                    col0 + qtl * 128 : col0 + qtl * 128 + 128],
                                acc[hp][0:64, cols], bcs[hp][:, cols],
                            )

            # ---- qc=0 attention: bf16 path ----
            def attention_bf16(fillers):
                ti = fi = 0
                ntiles = 2 * 4
                for mt in range(2):
                    acc = [
                        ps_acc.tile([128, 512], f32, tag="acc", name=f"accb{mt}{hp}")
                        for hp in range(2)
                    ]

                    def pv(kt, ex, off):
                        for hp in range(2):
                            h = 2 * mt + hp
                            nc.tensor.matmul(
                                acc[hp][0:65, off:512],
                                lhsT=vb_sb[:, kt, h * 65 : h * 65 + 65],
                                rhs=ex[:, hp, off:512],
                                start=(kt == 0), stop=(kt == 3),
                            )

                    pend = []
                    for kt in range(4):
                        off = 128 * kt if kt > 0 else 0
                        ps = ps_sc.tile([128, 2, 512], f32, tag="sc",
                                        name=f"scb{mt}{kt}")
                        for hp in range(2):
                            h = 2 * mt + hp
                            p0 = h * 32
                            for g in range(2):
                                nc.tensor.matmul(
                                    ps[:, hp, off:512],
                                    lhsT=ktb_sb[p0 : p0 + 32, g,
                                                kt * 128 : kt * 128 + 128],
                                    rhs=qtb_sb[p0 : p0 + 32, g, off:512],
                                    start=(g == 0), stop=(g == 1),
                                    tile_position=(p0, 0),
                                )
                        ex = exps.tile([128, 2, 512], bf16, tag="exb",
                                       name=f"exb{kt}")
                        nc.scalar.activation(
                            ex[:, :, off:512], ps[:, :, off:512], Exp, scale=0.125
                        )
                        # diagonal band mask (every qc0 tile is diagonal)
                        for hp in range(2):
                            nc.vector.tensor_mul(
                                ex[:, hp, off : off + 128],
                                ex[:, hp, off : off + 128],
                                maskb_sb,
                            )
                        ti += 1
                        want = min(len(fillers),
                                   ti * 3 * len(fillers) // (2 * ntiles))
                        while fi < want:
                            fillers[fi]()
                            fi += 1
                        if len(pend) == 2:
                            pv(*pend.pop(0))
                        pend.append((kt, ex, off))
                    for p in pend:
                        pv(*p)
                    normalize(0, mt, acc)
                while fi < len(fillers):
                    fillers[fi]()
                    fi += 1

            # ---- qc>=1 attention: fp8 DR path ----
            def attention_fp8(qc, fillers, pre_tail=None):
                nkt = 4 * (qc + 1)
                npair = nkt // 2
                ntiles = 2 * nkt
                ti = fi = 0
                for mt in range(2):
                    acc = [
                        ps_acc.tile([128, 512], f32, tag="acc",
                                    name=f"acc8{qc}{mt}{hp}")
                        for hp in range(2)
                    ]

                    def pv(m, ex, off0, off1):
                        # DR over the pair intersection [off1:512]; the
                        # earlier tile's extra band [off0:off1) as a plain
                        # fp8 matmul
                        for hp in range(2):
                            h = 2 * mt + hp
                            if off1 > off0:
                                nc.tensor.matmul(
                                    acc[hp][0:65, off0:off1],
                                    lhsT=v8_sb[:, 2 * m, h * 80 : h * 80 + 65],
                                    rhs=ex[:, hp, 0, off0:off1],
                                    start=False, stop=False,
                                )
                            nc.tensor.matmul(
                                acc[hp][0:65, off1:512],
                                lhsT=v8_sb[:, 2 * m : 2 * m + 2,
                                           h * 80 : h * 80 + 65],
                                rhs=ex[:, hp, :, off1:512],
                                start=(m == 0), stop=(m == npair - 1),
                                perf_mode=DR,
                            )

                    pend = []
                    for m in range(npair):
                        ex = exps.tile([128, 2, 2, 512], fp8, tag="ex8",
                                       name=f"ex8{m % 3}")
                        offs = []
                        for sl in range(2):
                            kt = 2 * m + sl
                            t = kt - 4 * qc
                            off = 128 * t if t > 0 else 0
                            offs.append(off)
                            ps = ps_sc.tile([128, 2, 512], f32, tag="sc",
                                            name=f"sc8{qc}{mt}{kt}")
                            for hp in range(2):
                                h = 2 * mt + hp
                                p0 = h * 32
                                nc.tensor.matmul(
                                    ps[:, hp, off:512],
                                    lhsT=kt8_sb[p0 : p0 + 32, :,
                                                kt * 128 : kt * 128 + 128],
                                    rhs=qt8_sb[p0 : p0 + 32, :,
                                               qc * 512 + off : (qc + 1) * 512],
                                    start=True, stop=True,
                                    perf_mode=DR,
                                    tile_position=(p0, 0),
                                )
                            nc.scalar.activation(
                                ex[:, :, sl, off:512], ps[:, :, off:512],
                                Exp, scale=0.125, bias=sh_sb,
                            )
                            if t >= 0:
                                # final stretch: DVE masks keep the critical
                                # exp->mask->PV chain off the Pool queue
                                meng = (nc.vector
                                        if mt == 1
                                        else nc.gpsimd)
                                for hp in range(2):
                                    meng.tensor_mul(
                                        ex[:, hp, sl, off : off + 128],
                                        ex[:, hp, sl, off : off + 128],
                                        mask8_sb,
                                    )
                            ti += 1
                            want = min(len(fillers),
                                   ti * 3 * len(fillers) // (2 * ntiles))
                            while fi < want:
                                fillers[fi]()
                                fi += 1
                        if len(pend) == 3:
                            pv(*pend.pop(0))
                        pend.append((m, ex, offs[0], offs[1]))
                    for p in pend:
                        pv(*p)
                    if mt == 1 and pre_tail is not None:
                        pre_tail()
                    normalize(qc, mt, acc, final=(qc == QC - 1 and mt == 1))
                while fi < len(fillers):
                    fillers[fi]()
                    fi += 1

            # ---- schedule ----
            for f in projb_qk():
                f()
            attention_bf16(projb_v() + proj8_fillers(1))
            attention_fp8(1, proj8_fillers(2) + outproj_fillers(0))
            attention_fp8(2, proj8_fillers(3) + outproj_fillers(1))

            # split epilogue: gr0 halves of out-proj(3) only need at8 gr 0
            # (mt=0, normalized before mt=1 runs), so they are issued right
            # after the mt=1 PV drain to keep PE busy through the final
            # normalize; gr1 halves + copies follow.
            ep_groups = [(qt, nh) for qt in range(12, 16) for nh in range(2)]
            ep_slots = {}

            def ep_phase_a():
                slots = [
                    ps_pj.tile([128, 512], f32, tag="pj", name="ep_pj0"),
                    ps_pj.tile([128, 512], f32, tag="pj", name="ep_pj1"),
                ]
                for i in range(2):
                    sc = ps_sc.tile([128, 2, 512], f32, tag="sc", name=f"ep_sc{i}")
                    slots += [sc[:, 0, :], sc[:, 1, :]]
                for i, (qt, nh) in enumerate(ep_groups[:6]):
                    ep_slots[(qt, nh)] = slots[i]
                    nc.tensor.matmul(
                        slots[i],
                        lhsT=at8_sb[:, 0, qt * 128 : qt * 128 + 128],
                        rhs=wo8_sb[:, 0, nh * 512 : nh * 512 + 512],
                        start=True, stop=False,
                    )

            attention_fp8(3, outproj_fillers(2), pre_tail=ep_phase_a)

            for i, (qt, nh) in enumerate(ep_groups):
                if (qt, nh) in ep_slots:
                    ps = ep_slots[(qt, nh)]
                    nc.tensor.matmul(
                        ps,
                        lhsT=at8_sb[:, 1, qt * 128 : qt * 128 + 128],
                        rhs=wo8_sb[:, 1, nh * 512 : nh * 512 + 512],
                        start=False, stop=True,
                    )
                else:
                    ps = ps_pj.tile([128, 512], f32, tag="pj", name=f"ep_y{qt}{nh}")
                    nc.tensor.matmul(
                        ps,
                        lhsT=at8_sb[:, :, qt * 128 : qt * 128 + 128],
                        rhs=wo8_sb[:, :, nh * 512 : nh * 512 + 512],
                        start=True, stop=True, perf_mode=DR,
                    )
                dst = y_sb[:, qt, nh * 512 : nh * 512 + 512]
                nc.scalar.copy(dst, ps)
                nc.sync.dma_start(
                    out=y[:, qt * E + nh * 512 : qt * E + nh * 512 + 512],
                    in_=dst,
                )

    nc.compile()
    return nc


def _get_program():
    global _prog
    if _prog is None:
        _prog = _build_program()
    return _prog


def _masks():
    import ml_dtypes

    k = np.arange(128)[:, None]
    q = np.arange(128)[None, :]
    m = np.ascontiguousarray(q >= k)
    return m.astype(ml_dtypes.bfloat16), m.astype(ml_dtypes.float8_e4m3)


def _gsplit_cols(bias=False):
    """Column permutation for the g-split feature packing.

    feature index f in [0,256): g = f//128, head = (f%128)//32,
    d = 32*g + f%32 -> source col = head*64 + d.
    """
    f = np.arange(256)
    g, r = f // 128, f % 128
    return (r // 32) * 64 + g * 32 + (f % 32)


def _core_inputs(x, Wq, bq, Wk, Wv, Wo, maskb, mask8, c):
    import ml_dtypes

    nbf = ml_dtypes.bfloat16
    nf8 = ml_dtypes.float8_e4m3
    b, g = divmod(c, 4)
    sl = slice(g * HD, (g + 1) * HD)
    xT = np.ascontiguousarray(x[b].T)  # [E, S]
    # [128, kt, cols] packs
    xTr = xT.reshape(KT, 128, S)
    xtb_p = np.ascontiguousarray(
        xTr[:, :, 0:512].transpose(1, 0, 2).reshape(128, KT * 512)
    )
    xt8_p = np.ascontiguousarray(
        xTr[:, :, 512:2048].reshape(KT, 128, 3, 512)
        .transpose(1, 2, 0, 3).reshape(128, 3 * KT * 512)
    )
    cols = _gsplit_cols()
    Wqs, Wks = Wq[:, sl][:, cols], Wk[:, sl][:, cols]  # [1024, 2*128] g-split
    # bf16, g-major g-split: [128, g, kt, 128]
    def packb(W):
        return np.ascontiguousarray(
            W.reshape(KT, 128, 2, 128).transpose(1, 2, 0, 3).reshape(128, -1)
        )
    # fp8: [128, (g,t)=8, i, 128]: a = g*4+t, rows (2t+i)*128
    def pack8(W):
        Wr = W.reshape(4, 2, 128, 2, 128)  # [t, i, p, g, f]
        return np.ascontiguousarray(
            Wr.transpose(2, 3, 0, 1, 4).reshape(128, 2 * 4 * 2 * 128)
        )
    wvr = Wv[:, sl].reshape(4, 2, 128, HD)  # [t, i, p, c]
    wv8_p = np.ascontiguousarray(wvr.transpose(2, 0, 1, 3).reshape(128, -1))
    wvb_p = np.ascontiguousarray(
        Wv[:, sl].reshape(KT, 128, HD).transpose(1, 0, 2).reshape(128, -1)
    )
    wo_p = np.ascontiguousarray(
        Wo[sl, :].reshape(2, 128, E).transpose(1, 0, 2).reshape(128, -1)
    )
    bqg = np.ascontiguousarray(bq[sl][_gsplit_cols()].reshape(2, 128).T)
    return {
        "xtb": xtb_p.astype(nbf),
        "xt8": xt8_p.astype(nf8),
        "wqb": packb(Wqs).astype(nbf),
        "wkb": packb(Wks).astype(nbf),
        "wq8": pack8(Wqs).astype(nf8),
        "wk8": pack8(Wks).astype(nf8),
        "wvb": wvb_p.astype(nbf),
        "wv8": wv8_p.astype(nf8),
        "wob": wo_p.astype(nbf),
        "wo8": wo_p.astype(nf8),
        "bqc": bqg.astype(np.float32),
        "maskb": maskb,
        "mask8": mask8,
    }


def _unpack_y(y_p):
    """[128, NQT*E] -> [S, E]"""
    return y_p.reshape(128, NQT, E).transpose(1, 0, 2).reshape(S, E)


def kernel(x, Wq, bq, Wk, bk, Wv, bv, Wo, bo, **_run_kwargs):
    from concourse.bass_utils import run_bass_kernel_spmd

    x = np.asarray(x, dtype=np.float32)
    Wq, bq = np.asarray(Wq, np.float32), np.asarray(bq, np.float32)
    Wk, bk = np.asarray(Wk, np.float32), np.asarray(bk, np.float32)
    Wv, bv = np.asarray(Wv, np.float32), np.asarray(bv, np.float32)
    Wo, bo = np.asarray(Wo, np.float32), np.asarray(bo, np.float32)

    nc = _get_program()
    maskb, mask8 = _masks()
    in_maps = [
        _core_inputs(x, Wq, bq, Wk, Wv, Wo, maskb, mask8, c) for c in range(NCORES)
    ]
    res = run_bass_kernel_spmd(nc, in_maps, list(range(NCORES)), **_run_kwargs)
    global LAST_RESULTS
    LAST_RESULTS = res
    parts = [_unpack_y(res.results[c]["y"].astype(np.float32)) for c in range(NCORES)]
    # bias identities: bk drops out of softmax; bv contributes bv @ Wo
    bias = bo + bv @ Wo
    out = np.empty((B, S, E), np.float32)
    for b in range(B):
        out[b] = parts[4 * b] + parts[4 * b + 1] + parts[4 * b + 2] + parts[4 * b + 3]
        out[b] += bias
    return out


# revision 39
# speedup vs baseline: 1.0144x; 1.0115x over previous
"""Causal self-attention (B=2, S=2048, E=1024, H=16, D=64) on 8 TRN2 cores.

Sharding: core c = (batch b = c//4, head-group g = c%4) owns batch b and
heads 4g..4g+3 (a 256-wide slice of the QKV projections / Wo rows).
Each core computes its partial out-projection y_c = attout_c @ Wo_c; the
host sums the 4 partials per batch and adds the folded bias (bk drops out
of softmax; bv contributes bv @ Wo since softmax weights sum to 1).

Row-dependent precision (validated: end-to-end rel err ~8e-3 vs 2e-2 gate):
  - queries 0-511 (qc=0): bf16 pipeline. Early rows have concentrated
    softmax and O(1)-magnitude attout, so they need bf16.
  - queries 512+ (qc=1..3): fp8e4m3 pipeline with DoubleRow (DR) matmuls.
    attout magnitude ~1.65/sqrt(L) for context length L, so fp8's ~4%
    relative noise stays far below the absolute error budget.
  DR matmul: lhsT [K,2,M] fp8, rhs [K,2,N] fp8 -> out[M,N] = sum_g
  lhsT[:,g].T @ rhs[:,g], at 0.5 PE cycles/output-column (4x bf16
  throughput per contraction element). DR free-dim group strides must be
  16-byte aligned -> V head slots padded to 80 cols.

Layouts:
  Q^T/K^T [128, 2, S]: partition p = head(p//32)*32 + d%32, free dim g =
  d//32 (the 32+32 d-split lets scores contract d=64 as DR [32,2]).
  V [128, rt, 320]: natural keys-on-partitions; per head h cols h*80..+63
  are V, col h*80+64 is ones (PV row 64 = softmax denominator).
  attout^T [128, 2, S]: partitions (h%2)*64+d, group h//2 -- matches the
  Wo row packing so out-proj is a single DR matmul per (qt, nh).

exp runs on ACT (the end-to-end bottleneck: ~58us of causal-area exp), in
[128, 2hp, 512-off] tiles; fp8-path exp folds scale 1/8 and bias -1.5
(softmax-invariant shift that keeps exp below e4m3's 240 max). The causal
mask multiply runs on GPSIMD for the fp8 path (SBUF-only engine), DVE for
the bf16 path. PE idle during exp is filled with the next chunk's
projections / previous chunks' out-projections (baseline's filler
interleave, engine queues are in-order).
"""

import numpy as np

B, S, E, H = 2, 2048, 1024, 16
D = E // H          # 64
NCORES = 8
HPC = 4             # heads per core
HD = HPC * D        # 256 cols per core
KT = E // 128       # 8 contraction tiles
QC = S // 512       # 4 query chunks
NQT = S // 128      # 16 row tiles
V8W = HPC * 80      # 320: fp8 V with 80-wide head slots (16B-aligned)
VBW = HPC * (D + 1)  # 260: bf16 V with ones column per head
SHIFT = 1.5         # exp bias: exp(s/8 - SHIFT), cancels in softmax

_prog = None
LAST_RESULTS = None


def _build_program():
    import concourse.mybir as mybir
    import concourse.tile as tile
    from concourse import bacc, library_config

    f32 = mybir.dt.float32
    bf16 = mybir.dt.bfloat16
    fp8 = mybir.dt.float8e4
    Exp = mybir.ActivationFunctionType.Exp
    DR = mybir.MatmulPerfMode.DoubleRow

    nc = bacc.Bacc(trn_type="TRN2", target_bir_lowering=False, debug=False)

    xtb = nc.dram_tensor("xtb", [128, KT * 512], bf16, kind="ExternalInput").ap()
    xt8 = nc.dram_tensor("xt8", [128, 3 * KT * 512], fp8, kind="ExternalInput").ap()
    wqb = nc.dram_tensor("wqb", [128, KT * 2 * 128], bf16, kind="ExternalInput").ap()
    wkb = nc.dram_tensor("wkb", [128, KT * 2 * 128], bf16, kind="ExternalInput").ap()
    wq8 = nc.dram_tensor("wq8", [128, 8 * 2 * 128], fp8, kind="ExternalInput").ap()
    wk8 = nc.dram_tensor("wk8", [128, 8 * 2 * 128], fp8, kind="ExternalInput").ap()
    wvb = nc.dram_tensor("wvb", [128, KT * 256], bf16, kind="ExternalInput").ap()
    wv8 = nc.dram_tensor("wv8", [128, 4 * 2 * 256], fp8, kind="ExternalInput").ap()
    wob = nc.dram_tensor("wob", [128, 2 * E], bf16, kind="ExternalInput").ap()
    wo8 = nc.dram_tensor("wo8", [128, 2 * E], fp8, kind="ExternalInput").ap()
    bq = nc.dram_tensor("bqc", [128, 2], f32, kind="ExternalInput").ap()
    # lower-triangular band mask (valid iff q_local >= k), bf16 + fp8 copies
    maskb = nc.dram_tensor("maskb", [128, 128], bf16, kind="ExternalInput").ap()
    mask8 = nc.dram_tensor("mask8", [128, 128], fp8, kind="ExternalInput").ap()
    y = nc.dram_tensor("y", [128, NQT * E], bf16, kind="ExternalOutput").ap()

    with tile.TileContext(nc) as tc:
        with (
            tc.tile_pool(name="consts", bufs=1) as consts,
            tc.tile_pool(name="exps", bufs=8) as exps,
            tc.tile_pool(name="small", bufs=4) as small,
            tc.tile_pool(name="ps_sc", bufs=2, space="PSUM") as ps_sc,
            tc.tile_pool(name="ps_pj", bufs=2, space="PSUM") as ps_pj,
            tc.tile_pool(name="ps_acc", bufs=2, space="PSUM") as ps_acc,
        ):
            # ---- SBUF constants; DMA issue order = need order ----
            xtb_sb = consts.tile([128, KT, 512], bf16)
            xt8_sb = consts.tile([128, 3, KT, 512], fp8)
            wqb_sb = consts.tile([128, 2, KT, 128], bf16)
            wkb_sb = consts.tile([128, 2, KT, 128], bf16)
            wq8_sb = consts.tile([128, 8, 2, 128], fp8)
            wk8_sb = consts.tile([128, 8, 2, 128], fp8)
            wvb_sb = consts.tile([128, KT, 256], bf16)
            wv8_sb = consts.tile([128, 4, 2, 256], fp8)
            wob_sb = consts.tile([128, 2, E], bf16)
            wo8_sb = consts.tile([128, 2, E], fp8)
            maskb_sb = consts.tile([128, 128], bf16)
            mask8_sb = consts.tile([128, 128], fp8)
            bq_sb = consts.tile([128, 2], f32)

            # All DMAs use flat 2D APs on both sides: one contiguous
            # descriptor per partition (>=512B elements avoid the 2x
            # small-transfer penalty; fewer descriptors saturate the bus).
            def load_xtb(quarter, eng=None):
                ks = slice(quarter * 2, quarter * 2 + 2)
                (eng or nc.sync).dma_start(
                    out=xtb_sb[:, ks].rearrange("p k c -> p (k c)"),
                    in_=xtb[:, ks.start * 512 : ks.stop * 512],
                )

            def load_w(dst, src, g, eng=None):
                # mt/g-major: one contiguous 2KB transfer unblocks the whole
                # half projection chain
                (eng or nc.sync).dma_start(
                    out=dst[:, g].rearrange("p k c -> p (k c)"),
                    in_=src[:, g * KT * 128 : (g + 1) * KT * 128],
                )

            def load_xt8(qx, eng=None):  # chunk index 0..3
                (eng or nc.sync).dma_start(
                    out=xt8_sb[:, qx].rearrange("p k c -> p (k c)"),
                    in_=xt8[:, qx * KT * 512 : (qx + 1) * KT * 512],
                )

            # DMA issue order: tiny consts, att0's projection gate
            # (weights early so both g-chains run as x quarters land), wvb
            # before the fp8 gate (att0's PV needs it at ~8us), then att1's
            # fp8 gate, then the tail. x tensors ride a second queue.
            nc.sync.dma_start(out=bq_sb, in_=bq)
            nc.sync.dma_start(out=maskb_sb, in_=maskb)
            load_w(wqb_sb, wqb, 0)
            load_xtb(0, eng=nc.scalar)
            load_xtb(1, eng=nc.scalar)
            load_w(wkb_sb, wkb, 0)
            load_w(wqb_sb, wqb, 1)
            load_w(wkb_sb, wkb, 1)
            load_xtb(2, eng=nc.scalar)
            load_xtb(3, eng=nc.scalar)
            nc.sync.dma_start(
                out=wvb_sb.rearrange("p k c -> p (k c)"), in_=wvb
            )
            nc.sync.dma_start(
                out=wq8_sb.rearrange("p a i c -> p (a i c)"), in_=wq8
            )
            nc.sync.dma_start(
                out=wk8_sb.rearrange("p a i c -> p (a i c)"), in_=wk8
            )
            load_xt8(0, eng=nc.scalar)
            nc.sync.dma_start(out=mask8_sb, in_=mask8)
            nc.sync.dma_start(
                out=wv8_sb.rearrange("p a i c -> p (a i c)"), in_=wv8
            )
            nc.gpsimd.load_library(library_config.attn)
            load_xt8(1, eng=nc.scalar)
            nc.sync.dma_start(out=wob_sb.rearrange("p g c -> p (g c)"), in_=wob)
            nc.sync.dma_start(out=wo8_sb.rearrange("p g c -> p (g c)"), in_=wo8)
            load_xt8(2, eng=nc.scalar)

            # force the Exp activation-table load off the critical path
            warm = small.tile([1, 8], f32, tag="warm", name="warm")
            nc.vector.memset(warm, 0.0)
            warm2 = small.tile([1, 8], f32, tag="warm", name="warm2")
            nc.scalar.activation(warm2, warm, Exp)
            # exp bias tile for the fp8 path
            sh_sb = consts.tile([128, 1], f32)
            nc.vector.memset(sh_sb, -SHIFT)

            # PE p-state warm-up: ~3us of continuous dummy matmuls while the
            # first DMAs land, so the real projections start at full clock
            wu = consts.tile([128, 512], bf16)
            nc.vector.memset(wu, 0.0)
            wups = ps_pj.tile([128, 512], f32, tag="pj", name="warmup")
            for i in range(8):
                nc.tensor.matmul(
                    wups, lhsT=wu[:, 0:128], rhs=wu,
                    start=(i == 0), stop=(i == 7),
                )


            # ---- persistent activations ----
            qtb_sb = consts.tile([128, 2, 512], bf16)   # chunk-0 Q, g-split
            ktb_sb = consts.tile([128, 2, 512], bf16)   # chunk-0 K, g-split
            qt8_sb = consts.tile([128, 2, S], fp8)      # g-split fp8 Q
            kt8_sb = consts.tile([128, 2, S], fp8)      # g-split fp8 K
            vb_sb = consts.tile([128, 4, VBW], bf16)    # chunk-0 V + ones
            v8_sb = consts.tile([128, NQT, V8W], fp8)   # fp8 V + ones
            nc.vector.memset(
                vb_sb.rearrange("p rt (h c) -> p rt h c", h=HPC)[:, :, :, D : D + 1],
                1.0,
            )
            nc.vector.memset(
                v8_sb.rearrange("p rt (h c) -> p rt h c", h=HPC)[:, :, :, D : D + 1],
                1.0,
            )
            atb_sb = consts.tile([128, 2, 512], bf16)   # chunk-0 attout^T
            at8_sb = consts.tile([128, 2, S], fp8)      # fp8 attout^T
            y_sb = consts.tile([128, NQT, E], bf16)

            # ---- projection fillers ----
            def projb_qk():
                """bf16 chunk-0 Q,K (gates the first exp); K bf16 copies go
                on ACT, which is idle during startup."""
                fs = []
                for g in range(2):
                    for w_sb, kind in ((wqb_sb, "q"), (wkb_sb, "k")):
                        box = {}

                        def h1(w_sb=w_sb, g=g, kind=kind, box=box):
                            ps = ps_pj.tile([128, 512], f32, tag="pj",
                                            name=f"pjb_{kind}{g}")
                            box["ps"] = ps
                            for kt in range(4):
                                nc.tensor.matmul(
                                    ps, lhsT=w_sb[:, g, kt, :], rhs=xtb_sb[:, kt],
                                    start=(kt == 0), stop=False,
                                )

                        def h2(w_sb=w_sb, g=g, kind=kind, box=box):
                            ps = box["ps"]
                            for kt in range(4, 8):
                                nc.tensor.matmul(
                                    ps, lhsT=w_sb[:, g, kt, :], rhs=xtb_sb[:, kt],
                                    start=False, stop=(kt == 7),
                                )
                            if kind == "q":
                                nc.vector.tensor_scalar_add(
                                    qtb_sb[:, g], ps, bq_sb[:, g : g + 1]
                                )
                            else:
                                nc.scalar.copy(ktb_sb[:, g], ps)
                                nc.vector.tensor_copy(kt8_sb[:, g, 0:512], ps)

                        fs += [h1, h2]
                return fs

            def projb_v():
                """bf16 chunk-0 V (runs as attention(0) filler; PV lags
                scores so vb arrives in time)."""
                fs = []
                for rl in range(4):
                    box = {}

                    def v1(rl=rl, box=box):
                        ps = ps_pj.tile([128, 512], f32, tag="pj", name=f"pjb_v{rl}")
                        box["ps"] = ps
                        for kt in range(4):
                            nc.tensor.matmul(
                                ps[:, 0:HD],
                                lhsT=xtb_sb[:, kt, rl * 128 : rl * 128 + 128],
                                rhs=wvb_sb[:, kt],
                                start=(kt == 0), stop=False,
                            )

                    def v2(rl=rl, box=box):
                        ps = box["ps"]
                        for kt in range(4, 8):
                            nc.tensor.matmul(
                                ps[:, 0:HD],
                                lhsT=xtb_sb[:, kt, rl * 128 : rl * 128 + 128],
                                rhs=wvb_sb[:, kt],
                                start=False, stop=(kt == 7),
                            )
                        psh = ps[:, 0:HD].rearrange("p (h c) -> p h c", h=HPC)
                        nc.vector.tensor_copy(
                            vb_sb[:, rl].rearrange("p (h c) -> p h c", h=HPC)[
                                :, :, 0:D
                            ],
                            psh,
                        )
                        nc.vector.tensor_copy(
                            v8_sb[:, rl].rearrange("p (h c) -> p h c", h=HPC)[
                                :, :, 0:D
                            ],
                            psh,
                        )

                    fs += [v1, v2]
                return fs

            def proj8_fillers(qc):
                """fp8 DR projections for chunk qc in 1..3."""
                fs = []
                for w_sb, kind in ((wq8_sb, "q"), (wk8_sb, "k")):
                    for g in range(2):

                        def f(w_sb=w_sb, g=g, kind=kind, qc=qc):
                            ps = ps_pj.tile([128, 512], f32, tag="pj",
                                            name=f"pj8_{kind}{qc}{g}")
                            for t in range(4):
                                nc.tensor.matmul(
                                    ps,
                                    lhsT=w_sb[:, g * 4 + t],
                                    rhs=xt8_sb[:, qc - 1, 2 * t : 2 * t + 2],
                                    start=(t == 0), stop=(t == 3),
                                    perf_mode=DR,
                                )
                            dst = qt8_sb if kind == "q" else kt8_sb
                            if kind == "q":
                                nc.vector.tensor_scalar_add(
                                    dst[:, g, qc * 512 : (qc + 1) * 512],
                                    ps, bq_sb[:, g : g + 1],
                                )
                            else:
                                nc.vector.tensor_copy(
                                    dst[:, g, qc * 512 : (qc + 1) * 512], ps
                                )

                        fs.append(f)
                for rl in range(4):

                    def fv(rl=rl, qc=qc):
                        rt = qc * 4 + rl
                        ps = ps_pj.tile([128, 512], f32, tag="pj", name=f"pj8_v{rt}")
                        for t in range(4):
                            nc.tensor.matmul(
                                ps[:, 0:HD],
                                lhsT=xt8_sb[:, qc - 1, 2 * t : 2 * t + 2,
                                            rl * 128 : rl * 128 + 128],
                                rhs=wv8_sb[:, t],
                                start=(t == 0), stop=(t == 3),
                                perf_mode=DR,
                            )
                        nc.vector.tensor_copy(
                            v8_sb[:, rt].rearrange("p (h c) -> p h c", h=HPC)[
                                :, :, 0:D
                            ],
                            ps[:, 0:HD].rearrange("p (h c) -> p h c", h=HPC),
                        )

                    fs.append(fv)
                return fs

            # ---- out-projection fillers ----
            def outproj_fillers(qc, use_act=False):
                fs = []
                for qtl in range(4):
                    qt = qc * 4 + qtl
                    for nh in range(2):

                        def f(qc=qc, qt=qt, qtl=qtl, nh=nh):
                            ps = ps_pj.tile([128, 512], f32, tag="pj",
                                            name=f"pj_y{qt}{nh}")
                            if qc == 0:
                                for kt2 in range(2):
                                    nc.tensor.matmul(
                                        ps,
                                        lhsT=atb_sb[:, kt2,
                                                    qtl * 128 : qtl * 128 + 128],
                                        rhs=wob_sb[:, kt2, nh * 512 : nh * 512 + 512],
                                        start=(kt2 == 0), stop=(kt2 == 1),
                                    )
                            else:
                                nc.tensor.matmul(
                                    ps,
                                    lhsT=at8_sb[:, :, qt * 128 : qt * 128 + 128],
                                    rhs=wo8_sb[:, :, nh * 512 : nh * 512 + 512],
                                    start=True, stop=True,
                                    perf_mode=DR,
                                )
                            dst = y_sb[:, qt, nh * 512 : nh * 512 + 512]
                            if use_act and nh == 0:
                                nc.scalar.copy(dst, ps)
                            else:
                                nc.vector.tensor_copy(dst, ps)
                            if nh == 1:
                                nc.sync.dma_start(
                                    out=y[:, qt * E : (qt + 1) * E],
                                    in_=y_sb[:, qt],
                                )

                        fs.append(f)
                return fs

            # ---- normalize one head-pair: attout = acc[0:64] / acc[64] ----
            def normalize(qc, mt, acc, final=False):
                dst = atb_sb if qc == 0 else at8_sb
                col0 = 0 if qc == 0 else qc * 512
                rc = small.tile([1, 2, 512], f32, tag="rc", name="rc")
                if not final:
                    nc.vector.reciprocal(out=rc[:, 0, :], in_=acc[0][64:65, :])
                    nc.vector.reciprocal(out=rc[:, 1, :], in_=acc[1][64:65, :])
                    for hp in range(2):
                        bc = small.tile([64, 512], f32, tag="bc", name=f"bc{hp}")
                        nc.gpsimd.partition_broadcast(
                            out_ap=bc, in_ap=rc[:, hp, :]
                        )
                        pb = hp * 64
                        nc.vector.tensor_mul(
                            dst[pb : pb + 64, mt, col0 : col0 + 512],
                            acc[hp][0:64, :], bc,
                        )
                else:
                    # final chunk: pipeline the whole normalize in 128-col
                    # pieces so the epilogue matmuls start ASAP
                    bcs = [small.tile([64, 512], f32, tag="bc", name=f"bc{hp}")
                           for hp in range(2)]
                    for qtl in range(4):
                        cols = slice(qtl * 128, qtl * 128 + 128)
                        for hp in range(2):
                            nc.vector.reciprocal(
                                out=rc[:, hp, cols], in_=acc[hp][64:65, cols]
                            )
                            nc.gpsimd.partition_broadcast(
                                out_ap=bcs[hp][:, cols], in_ap=rc[:, hp, cols]
                            )
                            pb = hp * 64
                            nc.vector.tensor_mul(
                                dst[pb : pb + 64, mt,
                                    col0 + qtl * 128 : col0 + qtl * 128 + 128],
                                acc[hp][0:64, cols], bcs[hp][:, cols],
                            )

            # ---- qc=0 attention: bf16 path ----
            def attention_bf16(fillers):
                ti = fi = 0
                ntiles = 2 * 4
                for mt in range(2):
                    acc = [
                        ps_acc.tile([128, 512], f32, tag="acc", name=f"accb{mt}{hp}")
                        for hp in range(2)
                    ]

                    def pv(kt, ex, off):
                        for hp in range(2):
                            h = 2 * mt + hp
                            nc.tensor.matmul(
                                acc[hp][0:65, off:512],
                                lhsT=vb_sb[:, kt, h * 65 : h * 65 + 65],
                                rhs=ex[:, hp, off:512],
                                start=(kt == 0), stop=(kt == 3),
                            )

                    pend = []
                    for kt in range(4):
                        off = 128 * kt if kt > 0 else 0
                        ps = ps_sc.tile([128, 2, 512], f32, tag="sc",
                                        name=f"scb{mt}{kt}")
                        for hp in range(2):
                            h = 2 * mt + hp
                            p0 = h * 32
                            for g in range(2):
                                nc.tensor.matmul(
                                    ps[:, hp, off:512],
                                    lhsT=ktb_sb[p0 : p0 + 32, g,
                                                kt * 128 : kt * 128 + 128],
                                    rhs=qtb_sb[p0 : p0 + 32, g, off:512],
                                    start=(g == 0), stop=(g == 1),
                                    tile_position=(p0, 0),
                                )
                        ex = exps.tile([128, 2, 512], bf16, tag="exb",
                                       name=f"exb{kt}")
                        nc.scalar.activation(
                            ex[:, :, off:512], ps[:, :, off:512], Exp, scale=0.125
                        )
                        # diagonal band mask (every qc0 tile is diagonal)
                        for hp in range(2):
                            nc.vector.tensor_mul(
                                ex[:, hp, off : off + 128],
                                ex[:, hp, off : off + 128],
                                maskb_sb,
                            )
                        ti += 1
                        want = min(len(fillers),
                                   ti * 3 * len(fillers) // (2 * ntiles))
                        while fi < want:
                            fillers[fi]()
                            fi += 1
                        if len(pend) == 2:
                            pv(*pend.pop(0))
                        pend.append((kt, ex, off))
                    for p in pend:
                        pv(*p)
                    normalize(0, mt, acc)
                while fi < len(fillers):
                    fillers[fi]()
                    fi += 1

            # ---- qc>=1 attention: fp8 DR path ----
            def attention_fp8(qc, fillers, pre_tail=None):
                nkt = 4 * (qc + 1)
                npair = nkt // 2
                ntiles = 2 * nkt
                ti = fi = 0
                for mt in range(2):
                    acc = [
                        ps_acc.tile([128, 512], f32, tag="acc",
                                    name=f"acc8{qc}{mt}{hp}")
                        for hp in range(2)
                    ]

                    def pv(m, ex, off0, off1):
                        # DR over the pair intersection [off1:512]; the
                        # earlier tile's extra band [off0:off1) as a plain
                        # fp8 matmul
                        for hp in range(2):
                            h = 2 * mt + hp
                            if off1 > off0:
                                nc.tensor.matmul(
                                    acc[hp][0:65, off0:off1],
                                    lhsT=v8_sb[:, 2 * m, h * 80 : h * 80 + 65],
                                    rhs=ex[:, hp, 0, off0:off1],
                                    start=False, stop=False,
                                )
                            nc.tensor.matmul(
                                acc[hp][0:65, off1:512],
                                lhsT=v8_sb[:, 2 * m : 2 * m + 2,
                                           h * 80 : h * 80 + 65],
                                rhs=ex[:, hp, :, off1:512],
                                start=(m == 0), stop=(m == npair - 1),
                                perf_mode=DR,
                            )

                    pend = []
                    for m in range(npair):
                        ex = exps.tile([128, 2, 2, 512], fp8, tag="ex8",
                                       name=f"ex8{m % 3}")
                        offs = []
                        for sl in range(2):
                            kt = 2 * m + sl
                            t = kt - 4 * qc
                            off = 128 * t if t > 0 else 0
                            offs.append(off)
                            ps = ps_sc.tile([128, 2, 512], f32, tag="sc",
                                            name=f"sc8{qc}{mt}{kt}")
                            for hp in range(2):
                                h = 2 * mt + hp
                                p0 = h * 32
                                nc.tensor.matmul(
                                    ps[:, hp, off:512],
                                    lhsT=kt8_sb[p0 : p0 + 32, :,
                                                kt * 128 : kt * 128 + 128],
                                    rhs=qt8_sb[p0 : p0 + 32, :,
                                               qc * 512 + off : (qc + 1) * 512],
                                    start=True, stop=True,
                                    perf_mode=DR,
                                    tile_position=(p0, 0),
                                )
                            nc.scalar.activation(
                                ex[:, :, sl, off:512], ps[:, :, off:512],
                                Exp, scale=0.125, bias=sh_sb,
                            )
                            if t >= 0:
                                # final stretch: DVE masks keep the critical
                                # exp->mask->PV chain off the Pool queue
                                meng = nc.vector
                                for hp in range(2):
                                    meng.tensor_mul(
                                        ex[:, hp, sl, off : off + 128],
                                        ex[:, hp, sl, off : off + 128],
                                        mask8_sb,
                                    )
                            ti += 1
                            want = min(len(fillers),
                                   ti * 3 * len(fillers) // (2 * ntiles))
                            while fi < want:
                                fillers[fi]()
                                fi += 1
                        if len(pend) == 3:
                            pv(*pend.pop(0))
                        pend.append((m, ex, offs[0], offs[1]))
                    for p in pend:
                        pv(*p)
                    if mt == 1 and pre_tail is not None:
                        pre_tail()
                    normalize(qc, mt, acc, final=(qc == QC - 1 and mt == 1))
                while fi < len(fillers):
                    fillers[fi]()
                    fi += 1

            # ---- schedule ----
            for f in projb_qk():
                f()
            attention_bf16(projb_v() + proj8_fillers(1))
            attention_fp8(1, proj8_fillers(2) + outproj_fillers(0))
            attention_fp8(2, proj8_fillers(3) + outproj_fillers(1))

            # split epilogue: gr0 halves of out-proj(3) only need at8 gr 0
            # (mt=0, normalized before mt=1 runs), so they are issued right
            # after the mt=1 PV drain to keep PE busy through the final
            # normalize; gr1 halves + copies follow.
            ep_groups = [(qt, nh) for qt in range(12, 16) for nh in range(2)]
            ep_slots = {}

            def ep_phase_a():
                slots = [
                    ps_pj.tile([128, 512], f32, tag="pj", name="ep_pj0"),
                    ps_pj.tile([128, 512], f32, tag="pj", name="ep_pj1"),
                ]
                for i in range(2):
                    sc = ps_sc.tile([128, 2, 512], f32, tag="sc", name=f"ep_sc{i}")
                    slots += [sc[:, 0, :], sc[:, 1, :]]
                for i, (qt, nh) in enumerate(ep_groups[:6]):
                    ep_slots[(qt, nh)] = slots[i]
                    nc.tensor.matmul(
                        slots[i],
                        lhsT=at8_sb[:, 0, qt * 128 : qt * 128 + 128],
                        rhs=wo8_sb[:, 0, nh * 512 : nh * 512 + 512],
                        start=True, stop=False,
                    )

            attention_fp8(3, outproj_fillers(2), pre_tail=ep_phase_a)

            for i, (qt, nh) in enumerate(ep_groups):
                if (qt, nh) in ep_slots:
                    ps = ep_slots[(qt, nh)]
                    nc.tensor.matmul(
                        ps,
                        lhsT=at8_sb[:, 1, qt * 128 : qt * 128 + 128],
                        rhs=wo8_sb[:, 1, nh * 512 : nh * 512 + 512],
                        start=False, stop=True,
                    )
                else:
                    ps = ps_pj.tile([128, 512], f32, tag="pj", name=f"ep_y{qt}{nh}")
                    nc.tensor.matmul(
                        ps,
                        lhsT=at8_sb[:, :, qt * 128 : qt * 128 + 128],
                        rhs=wo8_sb[:, :, nh * 512 : nh * 512 + 512],
                        start=True, stop=True, perf_mode=DR,
                    )
                dst = y_sb[:, qt, nh * 512 : nh * 512 + 512]
                nc.scalar.copy(dst, ps)
                nc.sync.dma_start(
                    out=y[:, qt * E + nh * 512 : qt * E + nh * 512 + 512],
                    in_=dst,
                )

    nc.compile()
    return nc


def _get_program():
    global _prog
    if _prog is None:
        _prog = _build_program()
    return _prog


def _masks():
    import ml_dtypes

    k = np.arange(128)[:, None]
    q = np.arange(128)[None, :]
    m = np.ascontiguousarray(q >= k)
    return m.astype(ml_dtypes.bfloat16), m.astype(ml_dtypes.float8_e4m3)


def _gsplit_cols(bias=False):
    """Column permutation for the g-split feature packing.

    feature index f in [0,256): g = f//128, head = (f%128)//32,
    d = 32*g + f%32 -> source col = head*64 + d.
    """
    f = np.arange(256)
    g, r = f // 128, f % 128
    return (r // 32) * 64 + g * 32 + (f % 32)


def _core_inputs(x, Wq, bq, Wk, Wv, Wo, maskb, mask8, c):
    import ml_dtypes

    nbf = ml_dtypes.bfloat16
    nf8 = ml_dtypes.float8_e4m3
    b, g = divmod(c, 4)
    sl = slice(g * HD, (g + 1) * HD)
    xT = np.ascontiguousarray(x[b].T)  # [E, S]
    # [128, kt, cols] packs
    xTr = xT.reshape(KT, 128, S)
    xtb_p = np.ascontiguousarray(
        xTr[:, :, 0:512].transpose(1, 0, 2).reshape(128, KT * 512)
    )
    xt8_p = np.ascontiguousarray(
        xTr[:, :, 512:2048].reshape(KT, 128, 3, 512)
        .transpose(1, 2, 0, 3).reshape(128, 3 * KT * 512)
    )
    cols = _gsplit_cols()
    Wqs, Wks = Wq[:, sl][:, cols], Wk[:, sl][:, cols]  # [1024, 2*128] g-split
    # bf16, g-major g-split: [128, g, kt, 128]
    def packb(W):
        return np.ascontiguousarray(
            W.reshape(KT, 128, 2, 128).transpose(1, 2, 0, 3).reshape(128, -1)
        )
    # fp8: [128, (g,t)=8, i, 128]: a = g*4+t, rows (2t+i)*128
    def pack8(W):
        Wr = W.reshape(4, 2, 128, 2, 128)  # [t, i, p, g, f]
        return np.ascontiguousarray(
            Wr.transpose(2, 3, 0, 1, 4).reshape(128, 2 * 4 * 2 * 128)
        )
    wvr = Wv[:, sl].reshape(4, 2, 128, HD)  # [t, i, p, c]
    wv8_p = np.ascontiguousarray(wvr.transpose(2, 0, 1, 3).reshape(128, -1))
    wvb_p = np.ascontiguousarray(
        Wv[:, sl].reshape(KT, 128, HD).transpose(1, 0, 2).reshape(128, -1)
    )
    wo_p = np.ascontiguousarray(
        Wo[sl, :].reshape(2, 128, E).transpose(1, 0, 2).reshape(128, -1)
    )
    bqg = np.ascontiguousarray(bq[sl][_gsplit_cols()].reshape(2, 128).T)
    return {
        "xtb": xtb_p.astype(nbf),
        "xt8": xt8_p.astype(nf8),
        "wqb": packb(Wqs).astype(nbf),
        "wkb": packb(Wks).astype(nbf),
        "wq8": pack8(Wqs).astype(nf8),
        "wk8": pack8(Wks).astype(nf8),
        "wvb": wvb_p.astype(nbf),
        "wv8": wv8_p.astype(nf8),
        "wob": wo_p.astype(nbf),
        "wo8": wo_p.astype(nf8),
        "bqc": bqg.astype(np.float32),
        "maskb": maskb,
        "mask8": mask8,
    }


def _unpack_y(y_p):
    """[128, NQT*E] -> [S, E]"""
    return y_p.reshape(128, NQT, E).transpose(1, 0, 2).reshape(S, E)


def kernel(x, Wq, bq, Wk, bk, Wv, bv, Wo, bo, **_run_kwargs):
    from concourse.bass_utils import run_bass_kernel_spmd

    x = np.asarray(x, dtype=np.float32)
    Wq, bq = np.asarray(Wq, np.float32), np.asarray(bq, np.float32)
    Wk, bk = np.asarray(Wk, np.float32), np.asarray(bk, np.float32)
    Wv, bv = np.asarray(Wv, np.float32), np.asarray(bv, np.float32)
    Wo, bo = np.asarray(Wo, np.float32), np.asarray(bo, np.float32)

    nc = _get_program()
    maskb, mask8 = _masks()
    in_maps = [
        _core_inputs(x, Wq, bq, Wk, Wv, Wo, maskb, mask8, c) for c in range(NCORES)
    ]
    res = run_bass_kernel_spmd(nc, in_maps, list(range(NCORES)), **_run_kwargs)
    global LAST_RESULTS
    LAST_RESULTS = res
    parts = [_unpack_y(res.results[c]["y"].astype(np.float32)) for c in range(NCORES)]
    # bias identities: bk drops out of softmax; bv contributes bv @ Wo
    bias = bo + bv @ Wo
    out = np.empty((B, S, E), np.float32)
    for b in range(B):
        out[b] = parts[4 * b] + parts[4 * b + 1] + parts[4 * b + 2] + parts[4 * b + 3]
        out[b] += bias
    return out
